# revision 1
# baseline (speedup 1.0000x reference)
"""Bass/Trainium2 kernel for nn_Attention_5909874999334.

Multi-head attention (B=2, N=2048, DIM=1024, H=16, DH=64) on 8 NeuronCores:
data-parallel over batch x tensor-parallel over heads (4 heads/core).
Each core computes a partial (N, DIM) output through its row-slice of Wout;
the host sums the 4 partials per batch (the "all-reduce after to_out").

Layout strategy (per core, transposed-flash):
  - qT/kT produced directly in (d, n) layout (lhsT=W chunk, rhs=xT chunk).
  - V produced in natural (n, d) layout (lhsT=xT chunk, rhs=Wv chunk),
    augmented with a ones column so the attn@V matmul also yields the
    softmax denominator for free.
  - simT[j, i] = kT.T @ qT per 128-row j-tile; softmax without max-
    subtraction (sim values are bounded ~ +-3); mask applied
    multiplicatively after exp with a host-precomputed combined
    (attn_mask | key_padding) validity mask in bf16.
  - normalization by 1/denom via gpsimd partition_broadcast + DVE mult,
    staggered into the following segment so it never head-of-line-blocks
    the DVE stream.
All matmuls run in bf16 at full PE rate (f32r pays a serialized internal
weight load on TRN2); inputs are cast host-side.
"""

import os
import sys

sys.path.insert(0, "/opt/trn_rl_repo")

import numpy as np
import ml_dtypes

import concourse.bass as bass
from concourse import bacc
import concourse.tile as tile
from concourse import mybir
from concourse.bass_utils import run_bass_kernel_spmd

F32 = mybir.dt.float32
F32R = mybir.dt.float32r
BF16 = mybir.dt.bfloat16

B, N, DIM, H, DH = 2, 2048, 1024, 16, 64
INNER = H * DH          # 1024
HC = 4                  # heads per core
E = HC * DH             # 256 inner cols per core
NT = N // 128           # 16 token tiles
CT = DIM // 128         # 8 contraction chunks
SCALE = DH ** -0.5

IB = 512                # i-block for the attention inner loop
NIB = N // IB


def build_nc():
    nc = bacc.Bacc()
    xt_ext = nc.declare_dram_parameter("xt", [DIM, N], BF16, isOutput=False)
    wq_ext = nc.declare_dram_parameter("wq", [DIM, E], BF16, isOutput=False)
    wk_ext = nc.declare_dram_parameter("wk", [DIM, E], BF16, isOutput=False)
    wv_ext = nc.declare_dram_parameter("wv", [DIM, E], BF16, isOutput=False)
    wout_ext = nc.declare_dram_parameter("wout", [HC, DH, DIM], BF16, isOutput=False)
    vld_ext = nc.declare_dram_parameter("validT", [N, N], BF16, isOutput=False)
    out_ext = nc.declare_dram_parameter("out", [N, DIM], F32, isOutput=True)

    Exp = mybir.ActivationFunctionType.Exp

    with tile.TileContext(nc) as tc:
        with (
            tc.tile_pool(name="persist", bufs=1) as pp,
            tc.tile_pool(name="vts", bufs=10) as vts,
            tc.tile_pool(name="pts", bufs=6) as pts,
            tc.tile_pool(name="norm", bufs=2) as nrm,
            tc.tile_pool(name="ostage", bufs=3) as ost,
            tc.tile_pool(name="psA", bufs=2, space="PSUM") as psA,
            tc.tile_pool(name="psB", bufs=4, space="PSUM") as psB,
        ):
            # ---- persistent SBUF tiles ----
            xt = pp.tile([128, CT, N], BF16, tag="xt")
            wq = pp.tile([128, CT, E], BF16, tag="wq")
            wk = pp.tile([128, CT, E], BF16, tag="wk")
            wv = pp.tile([128, CT, E], BF16, tag="wv")
            wo = pp.tile([DH, HC, DIM], BF16, tag="wo")
            qT = [pp.tile([128, N], BF16, tag=f"qT{i}", name=f"qT{i}") for i in range(2)]
            kT = [pp.tile([128, N], BF16, tag=f"kT{i}", name=f"kT{i}") for i in range(2)]
            vaug = pp.tile([128, NT, HC, DH + 1], BF16, tag="vaug")
            outT = pp.tile([DH, HC, N], BF16, tag="outT")

            nc.sync.dma_start(out=xt[:, 0:4, :],
                              in_=xt_ext[0:512, :].rearrange("(c p) n -> p c n", p=128))
            nc.sync.dma_start(out=xt[:, 4:8, :],
                              in_=xt_ext[512:1024, :].rearrange("(c p) n -> p c n", p=128))
            nc.sync.dma_start(out=wq, in_=wq_ext.rearrange("(c p) e -> p c e", p=128))
            nc.sync.dma_start(out=wk, in_=wk_ext.rearrange("(c p) e -> p c e", p=128))
            nc.sync.dma_start(out=wv, in_=wv_ext.rearrange("(c p) e -> p c e", p=128))
            nc.sync.dma_start(out=wo, in_=wout_ext.rearrange("h p f -> p h f"))
            nc.vector.memset(vaug[:, :, :, DH:DH + 1], 1.0)

            # PE warmup: ~6us of dummy matmuls while input DMAs land, so the
            # HAM clock-gate is at 8/8 when real matmuls start.
            wrm = pp.tile([64, 64], BF16, tag="wrm")
            nc.vector.memset(wrm, 0.0)
            wps = psA.tile([128, 2, IB], F32, tag="big", name="warmps")
            for wi in range(96):
                nc.tensor.matmul(wps[0:64, 0, 0:64], wrm, wrm,
                                 start=True, stop=True)

            # ---- phase 1: q/k projections -> qT/kT in (e, n) layout ----
            # mt order q0,k0,q1,k1 so head-pair 0 attention can start early.
            qk_groups = [
                [(0, wq, qT[0], 0), (1, wk, kT[0], 0)],
                [(2, wq, qT[1], 1), (3, wk, kT[1], 1)],
            ]

            def emit_qk(group):
                for mt, w_sb, dst, half in group:
                    for iq in range(4):
                        qkp = psB.tile([128, 512], F32, tag="med", name=f"qkp{mt}_{iq}")
                        for c in range(CT):
                            nc.tensor.matmul(
                                qkp,
                                w_sb[:, c, half * 128:half * 128 + 128],
                                xt[:, c, iq * 512:(iq + 1) * 512],
                                start=(c == 0), stop=(c == CT - 1),
                            )
                        nc.vector.tensor_copy(
                            out=dst[:, iq * 512:(iq + 1) * 512], in_=qkp)

            emit_qk(qk_groups[0])
            # v projection between the two qk groups: attention for head-pair
            # 0 can start while q1/k1 still project.
            for jt in range(NT):
                vp = psB.tile([128, E], F32, tag="med", name=f"vp{jt}")
                for c in range(CT):
                    nc.tensor.matmul(
                        vp, xt[:, c, jt * 128:(jt + 1) * 128], wv[:, c, :],
                        start=(c == 0), stop=(c == CT - 1),
                    )
                nc.vector.tensor_copy(
                    out=vaug[:, jt, :, 0:DH],
                    in_=vp.rearrange("p (h d) -> p h d", h=HC))
            emit_qk(qk_groups[1])

            _dead = []  # noqa: F841

            # ---- phase 3: attention, head-pair x i-block(512) ----
            def norm_piece(seg, hh, step, state):
                p_oa, p_ib, p_hp = seg
                p_isl = slice(p_ib * IB, (p_ib + 1) * IB)
                h = 2 * p_hp + hh
                if step == 0:
                    dn = nrm.tile([1, IB], F32, tag="dn", name=f"dn{p_ib}_{h}")
                    nc.vector.tensor_copy(out=dn, in_=p_oa[hh][DH:DH + 1, :])
                    rc = nrm.tile([1, IB], F32, tag="rc", name=f"rc{p_ib}_{h}")
                    nc.vector.reciprocal_approx_fast(out=rc, in_=dn)
                    rp = nrm.tile([DH, IB], F32, tag="rp", name=f"rp{p_ib}_{h}")
                    nc.gpsimd.partition_broadcast(rp, rc)
                    state[hh] = rp
                else:
                    nc.vector.tensor_mul(
                        out=outT[:, h, p_isl], in0=p_oa[hh][0:DH, :],
                        in1=state[hh])

            def emit_normalize(seg):
                st = {}
                for hh in range(2):
                    norm_piece(seg, hh, 0, st)
                    norm_piece(seg, hh, 1, st)

            def emit_fp(it, fh):
                fp = psB.tile([128, 512], F32, tag="med", name=f"fp{it}_{fh}")
                for h in range(HC):
                    nc.tensor.matmul(
                        fp,
                        outT[:, h, it * 128:(it + 1) * 128],
                        wo[:, h, fh * 512:(fh + 1) * 512],
                        start=(h == 0), stop=(h == HC - 1),
                    )
                ot = ost.tile([128, 512], F32, tag="ot", name=f"ot{it}_{fh}")
                nc.vector.tensor_copy(out=ot, in_=fp)
                nc.sync.dma_start(
                    out=out_ext[it * 128:(it + 1) * 128, fh * 512:(fh + 1) * 512],
                    in_=ot)

            fp_queue = []
            pending = None
            for ib in range(NIB):
                isl = slice(ib * IB, (ib + 1) * IB)
                for hp in range(2):
                    oa = [psB.tile([DH + 1, IB], F32, tag="med", name=f"oa{ib}_{hp}_{i}")
                          for i in range(2)]
                    norm_state = {}
                    for jt in range(NT):
                        vt = vts.tile([128, IB], BF16, tag="vt", name=f"vt{ib}_{hp}_{jt}")
                        nc.sync.dma_start(
                            out=vt, in_=vld_ext[jt * 128:(jt + 1) * 128, isl])
                        st = psA.tile([128, 2, IB], F32, tag="big", name=f"st{ib}_{hp}_{jt}")
                        for hh in range(2):
                            q_rows = slice(hh * 64, hh * 64 + 64)
                            nc.tensor.matmul(
                                st[:, hh, :],
                                kT[hp][q_rows, jt * 128:(jt + 1) * 128],
                                qT[hp][q_rows, isl],
                                start=True, stop=True,
                            )
                        pt = pts.tile([128, 2, IB], BF16, tag="pt", name=f"pt{ib}_{hp}_{jt}")
                        nc.scalar.activation(out=pt, in_=st, func=Exp, scale=SCALE)
                        ptm = pts.tile([128, 2, IB], BF16, tag="ptm", name=f"ptm{ib}_{hp}_{jt}")
                        nc.vector.tensor_mul(
                            out=ptm, in0=pt,
                            in1=vt.unsqueeze(1).broadcast_to((128, 2, IB)))
                        for hh in range(2):
                            nc.tensor.matmul(
                                oa[hh][:, :],
                                vaug[:, jt, 2 * hp + hh, :],
                                ptm[:, hh, :],
                                start=(jt == 0), stop=(jt == NT - 1),
                            )
                        if pending is not None and jt in (3, 5, 7, 9):
                            step = {3: (0, 0), 5: (0, 1), 7: (1, 0), 9: (1, 1)}[jt]
                            norm_piece(pending, step[0], step[1], norm_state)
                            if jt == 9:
                                pending = None
                    pending = (oa, ib, hp)


            if pending is not None:
                emit_normalize(pending)
                pending = None

            # ---- phase 4: out projection ----
            for it in range(NT):
                for fh in range(2):
                    emit_fp(it, fh)

    nc.finalize()
    return nc


_NC = None


def _get_nc():
    global _NC
    if _NC is None:
        _NC = build_nc()
    return _NC


def _install_trace_shim():
    """Provide antenv.axon_hooks for NTFF profiling under axon."""
    import types
    try:
        import antenv.axon_hooks  # noqa: F401
        return True
    except ImportError:
        pass
    try:
        from trn_agent_boot.trn_boot import _ntff_profile_via_ctypes
        hook = _ntff_profile_via_ctypes("/opt/axon/libaxon_pjrt.so")
    except Exception:
        return False
    if hook is None:
        return False
    mod = types.ModuleType("antenv.axon_hooks")
    mod.get_axon_ntff_profile_hook = lambda: hook
    sys.modules["antenv.axon_hooks"] = mod
    return True


def kernel(x, Wq, Wkv, Wout, attn_mask, key_padding_mask, _trace=False):
    x = np.asarray(x, dtype=np.float32)
    Wq = np.asarray(Wq, dtype=np.float32)
    Wkv = np.asarray(Wkv, dtype=np.float32)
    Wout = np.asarray(Wout, dtype=np.float32)
    attn_mask = np.asarray(attn_mask, dtype=bool)
    key_padding_mask = np.asarray(key_padding_mask, dtype=bool)

    nc = _get_nc()

    xT = [np.ascontiguousarray(x[b].T).astype(ml_dtypes.bfloat16) for b in range(B)]
    validT = []
    for b in range(B):
        v = ~(attn_mask.T | key_padding_mask[b][:, None])
        validT.append(v.astype(ml_dtypes.bfloat16))
    wq_s, wk_s, wv_s, wo_s = [], [], [], []
    for g in range(4):  # 4 head groups
        cols = slice(g * E, (g + 1) * E)
        wq_s.append(np.ascontiguousarray(Wq[:, cols]).astype(ml_dtypes.bfloat16))
        wk_s.append(np.ascontiguousarray(Wkv[:, cols]).astype(ml_dtypes.bfloat16))
        wv_s.append(np.ascontiguousarray(Wkv[:, INNER + g * E: INNER + (g + 1) * E]).astype(ml_dtypes.bfloat16))
        wo_s.append(np.ascontiguousarray(
            Wout[cols, :].reshape(HC, DH, DIM).astype(ml_dtypes.bfloat16)))

    in_maps = []
    for c in range(8):
        b, g = c // 4, c % 4
        in_maps.append({
            "xt": xT[b], "wq": wq_s[g], "wk": wk_s[g], "wv": wv_s[g],
            "wout": wo_s[g], "validT": validT[b],
        })

    if _trace:
        _install_trace_shim()
    res = run_bass_kernel_spmd(nc, in_maps, core_ids=list(range(8)), trace=_trace)

    out = np.empty((B, N, DIM), dtype=np.float32)
    for b in range(B):
        acc = res.results[4 * b]["out"].astype(np.float32)
        for g in range(1, 4):
            acc = acc + res.results[4 * b + g]["out"]
        out[b] = acc
    if _trace:
        kernel.last_exec_time_ns = res.exec_time_ns
    return out



# revision 3
# speedup vs baseline: 1.0042x; 1.0042x over previous
"""Bass/Trainium2 kernel for nn_Attention_5909874999334.

Multi-head attention (B=2, N=2048, DIM=1024, H=16, DH=64) on 8 NeuronCores:
data-parallel over batch x tensor-parallel over heads (4 heads/core).
Each core computes a partial (N, DIM) output through its row-slice of Wout;
the host sums the 4 partials per batch (the "all-reduce after to_out").

Layout strategy (per core, transposed-flash):
  - qT/kT produced directly in (d, n) layout (lhsT=W chunk, rhs=xT chunk).
  - V produced in natural (n, d) layout (lhsT=xT chunk, rhs=Wv chunk),
    augmented with a ones column so the attn@V matmul also yields the
    softmax denominator for free.
  - simT[j, i] = kT.T @ qT per 128-row j-tile; softmax without max-
    subtraction (sim values are bounded ~ +-3); mask applied
    multiplicatively after exp with a host-precomputed combined
    (attn_mask | key_padding) validity mask in bf16.
  - normalization by 1/denom via gpsimd partition_broadcast + DVE mult,
    staggered into the following segment so it never head-of-line-blocks
    the DVE stream.
All matmuls run in bf16 at full PE rate (f32r pays a serialized internal
weight load on TRN2); inputs are cast host-side.
"""

import os
import sys

sys.path.insert(0, "/opt/trn_rl_repo")

import numpy as np
import ml_dtypes

import concourse.bass as bass
from concourse import bacc
import concourse.tile as tile
from concourse import mybir
from concourse.bass_utils import run_bass_kernel_spmd

F32 = mybir.dt.float32
F32R = mybir.dt.float32r
BF16 = mybir.dt.bfloat16

B, N, DIM, H, DH = 2, 2048, 1024, 16, 64
INNER = H * DH          # 1024
HC = 4                  # heads per core
E = HC * DH             # 256 inner cols per core
NT = N // 128           # 16 token tiles
CT = DIM // 128         # 8 contraction chunks
SCALE = DH ** -0.5

IB = 512                # i-block for the attention inner loop
NIB = N // IB


def build_nc():
    nc = bacc.Bacc()
    xt_ext = nc.declare_dram_parameter("xt", [DIM, N], BF16, isOutput=False)
    wq_ext = nc.declare_dram_parameter("wq", [DIM, E], BF16, isOutput=False)
    wk_ext = nc.declare_dram_parameter("wk", [DIM, E], BF16, isOutput=False)
    wv_ext = nc.declare_dram_parameter("wv", [DIM, E], BF16, isOutput=False)
    wout_ext = nc.declare_dram_parameter("wout", [HC, DH, DIM], BF16, isOutput=False)
    vld_ext = nc.declare_dram_parameter("validT", [N, N], BF16, isOutput=False)
    out_ext = nc.declare_dram_parameter("out", [N, DIM], F32, isOutput=True)

    Exp = mybir.ActivationFunctionType.Exp

    with tile.TileContext(nc) as tc:
        with (
            tc.tile_pool(name="persist", bufs=1) as pp,
            tc.tile_pool(name="vts", bufs=10) as vts,
            tc.tile_pool(name="pts", bufs=6) as pts,
            tc.tile_pool(name="norm", bufs=2) as nrm,
            tc.tile_pool(name="ostage", bufs=3) as ost,
            tc.tile_pool(name="psA", bufs=2, space="PSUM") as psA,
            tc.tile_pool(name="psB", bufs=4, space="PSUM") as psB,
        ):
            # ---- persistent SBUF tiles ----
            xt = pp.tile([128, CT, N], BF16, tag="xt")
            wq = pp.tile([128, CT, E], BF16, tag="wq")
            wk = pp.tile([128, CT, E], BF16, tag="wk")
            wv = pp.tile([128, CT, E], BF16, tag="wv")
            wo = pp.tile([DH, HC, DIM], BF16, tag="wo")
            qT = [pp.tile([128, N], BF16, tag=f"qT{i}", name=f"qT{i}") for i in range(2)]
            kT = [pp.tile([128, N], BF16, tag=f"kT{i}", name=f"kT{i}") for i in range(2)]
            vaug = pp.tile([128, NT, HC, DH + 1], BF16, tag="vaug")
            outT = pp.tile([DH, HC, N], BF16, tag="outT")

            nc.sync.dma_start(out=xt[:, 0:4, :],
                              in_=xt_ext[0:512, :].rearrange("(c p) n -> p c n", p=128))
            nc.sync.dma_start(out=xt[:, 4:8, :],
                              in_=xt_ext[512:1024, :].rearrange("(c p) n -> p c n", p=128))
            nc.sync.dma_start(out=wq, in_=wq_ext.rearrange("(c p) e -> p c e", p=128))
            nc.sync.dma_start(out=wk, in_=wk_ext.rearrange("(c p) e -> p c e", p=128))
            nc.sync.dma_start(out=wv, in_=wv_ext.rearrange("(c p) e -> p c e", p=128))
            nc.sync.dma_start(out=wo, in_=wout_ext.rearrange("h p f -> p h f"))
            nc.vector.memset(vaug[:, :, :, DH:DH + 1], 1.0)

            # PE warmup: ~6us of dummy matmuls while input DMAs land, so the
            # HAM clock-gate is at 8/8 when real matmuls start.
            wrm = pp.tile([64, 64], BF16, tag="wrm")
            nc.vector.memset(wrm, 0.0)
            wps = psA.tile([128, 2, IB], F32, tag="big", name="warmps")
            for wi in range(96):
                nc.tensor.matmul(wps[0:64, 0, 0:64], wrm, wrm,
                                 start=True, stop=True)

            # ---- phase 1: q/k projections -> qT/kT in (e, n) layout ----
            # mt order q0,k0,q1,k1 so head-pair 0 attention can start early.
            qk_groups = [
                [(0, wq, qT[0], 0), (1, wk, kT[0], 0)],
                [(2, wq, qT[1], 1), (3, wk, kT[1], 1)],
            ]

            def emit_qk(group):
                for mt, w_sb, dst, half in group:
                    for iq in range(4):
                        qkp = psB.tile([128, 512], F32, tag="med", name=f"qkp{mt}_{iq}")
                        for c in range(CT):
                            nc.tensor.matmul(
                                qkp,
                                w_sb[:, c, half * 128:half * 128 + 128],
                                xt[:, c, iq * 512:(iq + 1) * 512],
                                start=(c == 0), stop=(c == CT - 1),
                            )
                        nc.vector.tensor_copy(
                            out=dst[:, iq * 512:(iq + 1) * 512], in_=qkp)

            emit_qk(qk_groups[0])
            # v projection between the two qk groups: attention for head-pair
            # 0 can start while q1/k1 still project.
            for jt in range(NT):
                vp = psB.tile([128, E], F32, tag="med", name=f"vp{jt}")
                for c in range(CT):
                    nc.tensor.matmul(
                        vp, xt[:, c, jt * 128:(jt + 1) * 128], wv[:, c, :],
                        start=(c == 0), stop=(c == CT - 1),
                    )
                nc.vector.tensor_copy(
                    out=vaug[:, jt, :, 0:DH],
                    in_=vp.rearrange("p (h d) -> p h d", h=HC))
            emit_qk(qk_groups[1])

            _dead = []  # noqa: F841

            # ---- phase 3: attention, head-pair x i-block(512) ----
            def norm_piece(seg, hh, step, state):
                p_oa, p_ib, p_hp = seg
                p_isl = slice(p_ib * IB, (p_ib + 1) * IB)
                h = 2 * p_hp + hh
                if step == 0:
                    dn = nrm.tile([1, IB], F32, tag="dn", name=f"dn{p_ib}_{h}")
                    nc.vector.tensor_copy(out=dn, in_=p_oa[hh][DH:DH + 1, :])
                    rc = nrm.tile([1, IB], F32, tag="rc", name=f"rc{p_ib}_{h}")
                    nc.vector.reciprocal_approx_fast(out=rc, in_=dn)
                    rp = nrm.tile([DH, IB], F32, tag="rp", name=f"rp{p_ib}_{h}")
                    nc.gpsimd.partition_broadcast(rp, rc)
                    state[hh] = rp
                else:
                    nc.vector.tensor_mul(
                        out=outT[:, h, p_isl], in0=p_oa[hh][0:DH, :],
                        in1=state[hh])

            def emit_normalize(seg):
                st = {}
                for hh in range(2):
                    norm_piece(seg, hh, 0, st)
                    norm_piece(seg, hh, 1, st)

            def emit_fp(it, fh):
                fp = psB.tile([128, 512], F32, tag="med", name=f"fp{it}_{fh}")
                for h in range(HC):
                    nc.tensor.matmul(
                        fp,
                        outT[:, h, it * 128:(it + 1) * 128],
                        wo[:, h, fh * 512:(fh + 1) * 512],
                        start=(h == 0), stop=(h == HC - 1),
                    )
                ot = ost.tile([128, 512], F32, tag="ot", name=f"ot{it}_{fh}")
                nc.vector.tensor_copy(out=ot, in_=fp)
                nc.sync.dma_start(
                    out=out_ext[it * 128:(it + 1) * 128, fh * 512:(fh + 1) * 512],
                    in_=ot)

            fp_queue = []
            pending = None
            for ib in range(NIB):
                isl = slice(ib * IB, (ib + 1) * IB)
                for hp in range(2):
                    oa = [psB.tile([DH + 1, IB], F32, tag="med", name=f"oa{ib}_{hp}_{i}")
                          for i in range(2)]
                    norm_state = {}
                    for jt in range(NT):
                        vt = vts.tile([128, IB], BF16, tag="vt", name=f"vt{ib}_{hp}_{jt}")
                        nc.sync.dma_start(
                            out=vt, in_=vld_ext[jt * 128:(jt + 1) * 128, isl])
                        st = psA.tile([128, 2, IB], F32, tag="big", name=f"st{ib}_{hp}_{jt}")
                        for hh in range(2):
                            q_rows = slice(hh * 64, hh * 64 + 64)
                            nc.tensor.matmul(
                                st[:, hh, :],
                                kT[hp][q_rows, jt * 128:(jt + 1) * 128],
                                qT[hp][q_rows, isl],
                                start=True, stop=True,
                                tile_position=(hh * 64, 0),
                            )
                        pt = pts.tile([128, 2, IB], BF16, tag="pt", name=f"pt{ib}_{hp}_{jt}")
                        nc.scalar.activation(out=pt, in_=st, func=Exp, scale=SCALE)
                        ptm = pts.tile([128, 2, IB], BF16, tag="ptm", name=f"ptm{ib}_{hp}_{jt}")
                        nc.vector.tensor_mul(
                            out=ptm, in0=pt,
                            in1=vt.unsqueeze(1).broadcast_to((128, 2, IB)))
                        for hh in range(2):
                            nc.tensor.matmul(
                                oa[hh][:, :],
                                vaug[:, jt, 2 * hp + hh, :],
                                ptm[:, hh, :],
                                start=(jt == 0), stop=(jt == NT - 1),
                            )
                        if pending is not None and jt in (3, 5, 7, 9):
                            step = {3: (0, 0), 5: (0, 1), 7: (1, 0), 9: (1, 1)}[jt]
                            norm_piece(pending, step[0], step[1], norm_state)
                            if jt == 9:
                                pending = None
                    pending = (oa, ib, hp)


            if pending is not None:
                emit_normalize(pending)
                pending = None

            # ---- phase 4: out projection ----
            for it in range(NT):
                for fh in range(2):
                    emit_fp(it, fh)

    nc.finalize()
    return nc


_NC = None


def _get_nc():
    global _NC
    if _NC is None:
        _NC = build_nc()
    return _NC


def _install_trace_shim():
    """Provide antenv.axon_hooks for NTFF profiling under axon."""
    import types
    try:
        import antenv.axon_hooks  # noqa: F401
        return True
    except ImportError:
        pass
    try:
        from trn_agent_boot.trn_boot import _ntff_profile_via_ctypes
        hook = _ntff_profile_via_ctypes("/opt/axon/libaxon_pjrt.so")
    except Exception:
        return False
    if hook is None:
        return False
    mod = types.ModuleType("antenv.axon_hooks")
    mod.get_axon_ntff_profile_hook = lambda: hook
    sys.modules["antenv.axon_hooks"] = mod
    return True


def kernel(x, Wq, Wkv, Wout, attn_mask, key_padding_mask, _trace=False):
    x = np.asarray(x, dtype=np.float32)
    Wq = np.asarray(Wq, dtype=np.float32)
    Wkv = np.asarray(Wkv, dtype=np.float32)
    Wout = np.asarray(Wout, dtype=np.float32)
    attn_mask = np.asarray(attn_mask, dtype=bool)
    key_padding_mask = np.asarray(key_padding_mask, dtype=bool)

    nc = _get_nc()

    xT = [np.ascontiguousarray(x[b].T).astype(ml_dtypes.bfloat16) for b in range(B)]
    validT = []
    for b in range(B):
        v = ~(attn_mask.T | key_padding_mask[b][:, None])
        validT.append(v.astype(ml_dtypes.bfloat16))
    wq_s, wk_s, wv_s, wo_s = [], [], [], []
    for g in range(4):  # 4 head groups
        cols = slice(g * E, (g + 1) * E)
        wq_s.append(np.ascontiguousarray(Wq[:, cols]).astype(ml_dtypes.bfloat16))
        wk_s.append(np.ascontiguousarray(Wkv[:, cols]).astype(ml_dtypes.bfloat16))
        wv_s.append(np.ascontiguousarray(Wkv[:, INNER + g * E: INNER + (g + 1) * E]).astype(ml_dtypes.bfloat16))
        wo_s.append(np.ascontiguousarray(
            Wout[cols, :].reshape(HC, DH, DIM).astype(ml_dtypes.bfloat16)))

    in_maps = []
    for c in range(8):
        b, g = c // 4, c % 4
        in_maps.append({
            "xt": xT[b], "wq": wq_s[g], "wk": wk_s[g], "wv": wv_s[g],
            "wout": wo_s[g], "validT": validT[b],
        })

    if _trace:
        _install_trace_shim()
    res = run_bass_kernel_spmd(nc, in_maps, core_ids=list(range(8)), trace=_trace)

    out = np.empty((B, N, DIM), dtype=np.float32)
    for b in range(B):
        acc = res.results[4 * b]["out"].astype(np.float32)
        for g in range(1, 4):
            acc = acc + res.results[4 * b + g]["out"]
        out[b] = acc
    if _trace:
        kernel.last_exec_time_ns = res.exec_time_ns
    return out



# revision 11
# speedup vs baseline: 1.0743x; 1.0699x over previous
"""Bass/Trainium2 kernel for nn_Attention_5909874999334.

Multi-head attention (B=2, N=2048, DIM=1024, H=16, DH=64) on 8 NeuronCores:
data-parallel over batch x tensor-parallel over heads (4 heads/core).
Each core computes a partial (N, DIM) output through its row-slice of Wout;
the host sums the 4 partials per batch (the "all-reduce after to_out").

Layout strategy (per core, transposed-flash):
  - qT/kT produced directly in (d, n) layout (lhsT=W chunk, rhs=xT chunk).
  - V produced in natural (n, d) layout, augmented with a ones column so the
    attn@V matmul also yields the softmax denominator for free.
  - simT[j, i] = kT.T @ qT per 128-row j-tile (two heads concurrently on
    disjoint 64-row PE tiles); softmax without max-subtraction; mask applied
    multiplicatively after exp with a host-precomputed combined validity
    mask in bf16 (batched over jt pairs on the DVE).
  - normalization by 1/denom via gpsimd partition_broadcast + DVE mult,
    staggered into the following segment.
  - out-projection pairs two heads along the contraction (outP rows 0-63 =
    even head dims, 64-127 = odd head) so each output tile needs only 2
    matmuls; odd-head normalized values reach partitions 64-127 via a
    SBUF->SBUF DMA hop.
Emission is hp-major: all i-blocks for head-pair 0, then head-pair 1, with
v-proj / remaining q/k projections / out-proj matmuls injected into the
ACT-bound attention stream so the PE fills its exp-wait bubbles.
All matmuls run in bf16 at full PE rate; inputs are cast host-side.
"""

import os
import sys

sys.path.insert(0, "/opt/trn_rl_repo")

import numpy as np
import ml_dtypes

import concourse.bass as bass
from concourse import bacc
import concourse.tile as tile
from concourse import mybir
from concourse.bass_utils import run_bass_kernel_spmd

F32 = mybir.dt.float32
BF16 = mybir.dt.bfloat16

B, N, DIM, H, DH = 2, 2048, 1024, 16, 64
INNER = H * DH          # 1024
HC = 4                  # heads per core
E = HC * DH             # 256 inner cols per core
NT = N // 128           # 16 token tiles
CT = DIM // 128         # 8 contraction chunks
SCALE = DH ** -0.5

IB = 512                # i-block for the attention inner loop
NIB = N // IB


def build_nc():
    nc = bacc.Bacc()
    xt_ext = nc.declare_dram_parameter("xt", [DIM, N], BF16, isOutput=False)
    wq_ext = nc.declare_dram_parameter("wq", [DIM, E], BF16, isOutput=False)
    wk_ext = nc.declare_dram_parameter("wk", [DIM, E], BF16, isOutput=False)
    wv_ext = nc.declare_dram_parameter("wv", [DIM, E], BF16, isOutput=False)
    # Wout relaid as [128, 2, DIM]: row s*64+d, pair g holds
    # Wout[(2g+s)*64 + d, :] so one K=128 matmul sums a head pair.
    wout_ext = nc.declare_dram_parameter("wout", [128, 2, DIM], BF16, isOutput=False)
    vld_ext = nc.declare_dram_parameter("validT", [N, N], BF16, isOutput=False)
    out_ext = nc.declare_dram_parameter("out", [N, DIM], F32, isOutput=True)

    Exp = mybir.ActivationFunctionType.Exp

    with tile.TileContext(nc) as tc:
        with (
            tc.tile_pool(name="persist", bufs=1) as pp,
            tc.tile_pool(name="vts", bufs=5) as vts,
            tc.tile_pool(name="pts", bufs=3) as pts,
            tc.tile_pool(name="norm", bufs=2) as nrm,
            tc.tile_pool(name="ostage", bufs=3) as ost,
            tc.tile_pool(name="psA", bufs=2, space="PSUM") as psA,
            tc.tile_pool(name="psB", bufs=4, space="PSUM") as psB,
        ):
            # ---- persistent SBUF tiles ----
            xt = pp.tile([128, CT, N], BF16, tag="xt")
            wq = pp.tile([128, CT, E], BF16, tag="wq")
            wk = pp.tile([128, CT, E], BF16, tag="wk")
            wv = pp.tile([128, CT, E], BF16, tag="wv")
            woP = pp.tile([128, 2, DIM], BF16, tag="woP")
            qT = [pp.tile([128, N], BF16, tag=f"qT{i}", name=f"qT{i}") for i in range(2)]
            kT = [pp.tile([128, N], BF16, tag=f"kT{i}", name=f"kT{i}") for i in range(2)]
            vaug = pp.tile([128, NT, HC, DH + 1], BF16, tag="vaug")
            outP = pp.tile([128, 2, N], BF16, tag="outP")

            # input DMAs; xt chunked by token-quarter so the first k0
            # projection block can start after ~1/4 of xt lands.
            nc.sync.dma_start(out=wk, in_=wk_ext.rearrange("(c p) e -> p c e", p=128))
            nc.sync.dma_start(out=wq, in_=wq_ext.rearrange("(c p) e -> p c e", p=128))
            for q in range(4):
                qs = slice(q * 512, (q + 1) * 512)
                nc.sync.dma_start(
                    out=xt[:, :, qs],
                    in_=xt_ext[:, qs].rearrange("(c p) n -> p c n", p=128))
            nc.sync.dma_start(out=wv, in_=wv_ext.rearrange("(c p) e -> p c e", p=128))
            nc.sync.dma_start(out=woP, in_=wout_ext[:, :, :])
            nc.vector.memset(vaug[:, :, :, DH:DH + 1], 1.0)

            # PE warmup: ~5us of dummy matmuls while input DMAs land, so the
            # HAM clock-gate is at 8/8 when real matmuls start.
            wrm = pp.tile([64, 64], BF16, tag="wrm")
            nc.vector.memset(wrm, 0.0)
            wps = psA.tile([128, 2, IB], F32, tag="big", name="warmps")
            for wi in range(96):
                nc.tensor.matmul(wps[0:64, 0, 0:64], wrm, wrm,
                                 start=True, stop=True)

            # ---- q/k projection blocks (emitted piecemeal) ----
            def emit_qk_block(w_sb, dst, half, iq):
                qkp = psB.tile([128, 512], F32, tag="med",
                               name=f"qkp{half}_{iq}_{dst.name if hasattr(dst,'name') else 0}")
                for c in range(CT):
                    nc.tensor.matmul(
                        qkp,
                        w_sb[:, c, half * 128:half * 128 + 128],
                        xt[:, c, iq * 512:(iq + 1) * 512],
                        start=(c == 0), stop=(c == CT - 1),
                    )
                nc.vector.tensor_copy(
                    out=dst[:, iq * 512:(iq + 1) * 512], in_=qkp)

            def emit_v(jt):
                vp = psB.tile([128, E], F32, tag="med", name=f"vp{jt}")
                for c in range(CT):
                    nc.tensor.matmul(
                        vp, xt[:, c, jt * 128:(jt + 1) * 128], wv[:, c, :],
                        start=(c == 0), stop=(c == CT - 1),
                    )
                nc.vector.tensor_copy(
                    out=vaug[:, jt, :, 0:DH],
                    in_=vp.rearrange("p (h d) -> p h d", h=HC))

            def emit_outproj_unit(ib, it, fh):
                fp = psB.tile([128, 512], F32, tag="med", name=f"fp{it}_{fh}")
                for g in range(2):
                    nc.tensor.matmul(
                        fp,
                        outP[:, g, it * 128:(it + 1) * 128],
                        woP[:, g, fh * 512:(fh + 1) * 512],
                        start=(g == 0), stop=(g == 1),
                    )
                ot = ost.tile([128, 512], F32, tag="ot", name=f"ot{it}_{fh}")
                nc.vector.tensor_copy(out=ot, in_=fp)
                nc.sync.dma_start(
                    out=out_ext[it * 128:(it + 1) * 128, fh * 512:(fh + 1) * 512],
                    in_=ot)

            # staggered normalization of a finished segment's accumulators
            def norm_piece(seg, step, state):
                p_oa, p_ib, p_hp = seg
                p_isl = slice(p_ib * IB, (p_ib + 1) * IB)
                hh, sub = divmod(step, 3)
                if sub == 0:
                    dn = nrm.tile([1, IB], F32, tag="dn", name=f"dn{p_ib}_{p_hp}_{hh}")
                    nc.vector.tensor_copy(out=dn, in_=p_oa[hh][DH:DH + 1, :])
                    rc = nrm.tile([1, IB], F32, tag="rc", name=f"rc{p_ib}_{p_hp}_{hh}")
                    nc.vector.reciprocal_approx_fast(out=rc, in_=dn)
                    state[("rc", hh)] = rc
                elif sub == 1:
                    rp = nrm.tile([DH, IB], F32, tag="rp", name=f"rp{p_ib}_{p_hp}_{hh}")
                    nc.gpsimd.partition_broadcast(rp, state[("rc", hh)])
                    state[("rp", hh)] = rp
                else:
                    if hh == 0:
                        nc.vector.tensor_mul(
                            out=outP[0:DH, p_hp, p_isl], in0=p_oa[0][0:DH, :],
                            in1=state[("rp", 0)])
                    else:
                        stg = nrm.tile([DH, IB], BF16, tag="stg",
                                       name=f"stg{p_ib}_{p_hp}")
                        nc.vector.tensor_mul(
                            out=stg, in0=p_oa[1][0:DH, :], in1=state[("rp", 1)])
                        nc.sync.dma_start(
                            out=outP[DH:128, p_hp, p_isl], in_=stg)

            # injection schedule per segment index 0..7 (hp-major: seg = hp*4+ib)
            # v-proj for segment 0 is handled at pair-top (just-in-time for
            # the avs); everything else spreads over jt slots 6..15, after
            # the pending norm of the previous segment has freed its oa
            # slots and written its outP range.
            inject = {k: [] for k in range(8)}
            inject[0] = [("qk", "q0", 1)]
            inject[1] = [("qk", "k1", 0), ("qk", "k1", 1), ("qk", "k1", 2),
                         ("qk", "k1", 3), ("qk", "q0", 2)]
            inject[2] = [("qk", "q1", 0), ("qk", "q0", 3)]
            inject[3] = [("qk", "q1", 1)]
            inject[4] = [("qk", "q1", 2)]
            inject[5] = [("op", 0), ("qk", "q1", 3)]
            inject[6] = [("op", 1)]
            inject[7] = [("op", 2)]

            qk_map = {
                "q0": (wq, qT[0], 0), "k0": (wk, kT[0], 0),
                "q1": (wq, qT[1], 1), "k1": (wk, kT[1], 1),
            }

            # prologue: k0 fully + q0 block 0 -> segment 0 can start
            for iq in range(4):
                emit_qk_block(wk, kT[0], 0, iq)
            emit_qk_block(wq, qT[0], 0, 0)

            pending = None
            pending_state = {}
            pending_step = 0

            for seg in range(8):
                hp, ib = divmod(seg, 4)
                isl = slice(ib * IB, (ib + 1) * IB)
                inj = list(inject[seg])
                # expand "op" entries into 8 outproj units, "v" kept as is
                units = []
                for item in inj:
                    if item[0] == "op":
                        oib = item[1]
                        for it in range(oib * 4, oib * 4 + 4):
                            for fh in range(2):
                                units.append(("op1", oib, it, fh))
                    else:
                        units.append(item)
                # spread units over jt slots 6..15 (after pending-norm)
                slots = {}
                for u_i, u in enumerate(units):
                    slot = 6 + min(9, (u_i * 10) // max(1, len(units)))
                    slots.setdefault(slot, []).append(u)

                oa = [psB.tile([DH + 1, IB], F32, tag="med",
                               name=f"oa{seg}_{i}") for i in range(2)]
                for jp in range(NT // 2):  # 8 jt pairs
                    je, jo = 2 * jp, 2 * jp + 1
                    if seg == 0:
                        # just-in-time v projection: avs of this pair need
                        # vaug[:, je/jo] and read it in program order.
                        emit_v(je)
                        emit_v(jo)
                    vt2 = vts.tile([128, 2, IB], BF16, tag="vt",
                                   name=f"vt{seg}_{jp}")
                    nc.sync.dma_start(
                        out=vt2[:, 0, :], in_=vld_ext[je * 128:(je + 1) * 128, isl])
                    nc.sync.dma_start(
                        out=vt2[:, 1, :], in_=vld_ext[jo * 128:(jo + 1) * 128, isl])
                    pt2 = pts.tile([128, 2, 2, IB], BF16, tag="pt",
                                   name=f"pt{seg}_{jp}")
                    ptm2 = pts.tile([128, 2, 2, IB], BF16, tag="ptm",
                                    name=f"ptm{seg}_{jp}")
                    for jt in (je, jo):
                        st = psA.tile([128, 2, IB], F32, tag="big",
                                      name=f"st{seg}_{jt}")
                        for hh in range(2):
                            q_rows = slice(hh * 64, hh * 64 + 64)
                            nc.tensor.matmul(
                                st[:, hh, :],
                                kT[hp][q_rows, jt * 128:(jt + 1) * 128],
                                qT[hp][q_rows, isl],
                                start=True, stop=True,
                                tile_position=(hh * 64, 0),
                            )
                        nc.scalar.activation(out=pt2[:, jt % 2, :, :], in_=st,
                                             func=Exp, scale=SCALE)
                    nc.vector.tensor_mul(
                        out=ptm2, in0=pt2,
                        in1=vt2.unsqueeze(2).broadcast_to((128, 2, 2, IB)))
                    for jt in (je, jo):
                        for hh in range(2):
                            nc.tensor.matmul(
                                oa[hh][:, :],
                                vaug[:, jt, 2 * hp + hh, :],
                                ptm2[:, jt % 2, hh, :],
                                start=(jt == 0), stop=(jt == NT - 1),
                            )
                    # staggered pieces: pending norm first, then injections
                    for jt in (je, jo):
                        if pending is not None and pending_step < 6:
                            norm_piece(pending, pending_step, pending_state)
                            pending_step += 1
                            if pending_step == 6:
                                pending = None
                        for u in slots.get(jt, []):
                            if u[0] == "v":
                                emit_v(u[1])
                            elif u[0] == "qk":
                                w_sb, dst, half = qk_map[u[1]]
                                emit_qk_block(w_sb, dst, half, u[2])
                            elif u[0] == "op1":
                                emit_outproj_unit(u[1], u[2], u[3])
                pending = (oa, ib, hp)
                pending_state = {}
                pending_step = 0

            # tail: final segment's norm + last outproj block
            while pending_step < 6:
                norm_piece(pending, pending_step, pending_state)
                pending_step += 1
            for it in range(12, 16):
                for fh in range(2):
                    emit_outproj_unit(3, it, fh)

    nc.finalize()
    return nc


_NC = None


def _get_nc():
    global _NC
    if _NC is None:
        _NC = build_nc()
    return _NC


def _install_trace_shim():
    """Provide antenv.axon_hooks for NTFF profiling under axon."""
    import types
    try:
        import antenv.axon_hooks  # noqa: F401
        return True
    except ImportError:
        pass
    try:
        from trn_agent_boot.trn_boot import _ntff_profile_via_ctypes
        hook = _ntff_profile_via_ctypes("/opt/axon/libaxon_pjrt.so")
    except Exception:
        return False
    if hook is None:
        return False
    mod = types.ModuleType("antenv.axon_hooks")
    mod.get_axon_ntff_profile_hook = lambda: hook
    sys.modules["antenv.axon_hooks"] = mod
    return True


def kernel(x, Wq, Wkv, Wout, attn_mask, key_padding_mask, _trace=False):
    x = np.asarray(x, dtype=np.float32)
    Wq = np.asarray(Wq, dtype=np.float32)
    Wkv = np.asarray(Wkv, dtype=np.float32)
    Wout = np.asarray(Wout, dtype=np.float32)
    attn_mask = np.asarray(attn_mask, dtype=bool)
    key_padding_mask = np.asarray(key_padding_mask, dtype=bool)

    nc = _get_nc()

    xT = [np.ascontiguousarray(x[b].T).astype(ml_dtypes.bfloat16) for b in range(B)]
    validT = []
    for b in range(B):
        v = ~(attn_mask.T | key_padding_mask[b][:, None])
        validT.append(v.astype(ml_dtypes.bfloat16))
    wq_s, wk_s, wv_s, wo_s = [], [], [], []
    for g in range(4):  # 4 head groups
        cols = slice(g * E, (g + 1) * E)
        wq_s.append(np.ascontiguousarray(Wq[:, cols]).astype(ml_dtypes.bfloat16))
        wk_s.append(np.ascontiguousarray(Wkv[:, cols]).astype(ml_dtypes.bfloat16))
        wv_s.append(np.ascontiguousarray(Wkv[:, INNER + g * E: INNER + (g + 1) * E]).astype(ml_dtypes.bfloat16))
        # [128, 2, DIM]: row s*64+d of pair g = Wout[(2g+s)*64+d, :]
        w4 = Wout[cols, :].reshape(2, 2, DH, DIM)  # [g, s, d, f]
        wo_s.append(np.ascontiguousarray(
            w4.transpose(1, 2, 0, 3).reshape(128, 2, DIM)).astype(ml_dtypes.bfloat16))

    in_maps = []
    for c in range(8):
        b, g = c // 4, c % 4
        in_maps.append({
            "xt": xT[b], "wq": wq_s[g], "wk": wk_s[g], "wv": wv_s[g],
            "wout": wo_s[g], "validT": validT[b],
        })

    if _trace:
        _install_trace_shim()
    res = run_bass_kernel_spmd(nc, in_maps, core_ids=list(range(8)), trace=_trace)

    out = np.empty((B, N, DIM), dtype=np.float32)
    for b in range(B):
        acc = res.results[4 * b]["out"].astype(np.float32)
        for g in range(1, 4):
            acc = acc + res.results[4 * b + g]["out"]
        out[b] = acc
    if _trace:
        kernel.last_exec_time_ns = res.exec_time_ns
    return out


# revision 19
# speedup vs baseline: 1.0984x; 1.0224x over previous
"""Bass/Trainium2 kernel for nn_Attention_5909874999334.

Multi-head attention (B=2, N=2048, DIM=1024, H=16, DH=64) on 8 NeuronCores:
data-parallel over batch x tensor-parallel over heads (4 heads/core).
Each core computes a partial (N, DIM) output through its row-slice of Wout;
the host sums the 4 partials per batch (the "all-reduce after to_out").

Layout strategy (per core, transposed-flash):
  - qT/kT produced directly in (d, n) layout (lhsT=W chunk, rhs=xT chunk).
  - V produced in natural (n, d) layout, augmented with a ones column so the
    attn@V matmul also yields the softmax denominator for free.
  - simT[j, i] = kT.T @ qT per 128-row j-tile (two heads concurrently on
    disjoint 64-row PE tiles); softmax without max-subtraction; mask applied
    multiplicatively after exp with a host-precomputed combined validity
    mask in bf16 (batched over jt pairs on the DVE).
  - normalization by 1/denom via gpsimd partition_broadcast + DVE mult,
    staggered into the following segment.
  - out-projection pairs two heads along the contraction (outP rows 0-63 =
    even head dims, 64-127 = odd head) so each output tile needs only 2
    matmuls; odd-head normalized values reach partitions 64-127 via a
    SBUF->SBUF DMA hop.
Emission is hp-major: all i-blocks for head-pair 0, then head-pair 1, with
v-proj / remaining q/k projections / out-proj matmuls injected into the
ACT-bound attention stream so the PE fills its exp-wait bubbles.
All matmuls run in bf16 at full PE rate; inputs are cast host-side.
"""

import os
import sys

sys.path.insert(0, "/opt/trn_rl_repo")

import numpy as np
import ml_dtypes

import concourse.bass as bass
from concourse import bacc
import concourse.tile as tile
from concourse import mybir
from concourse.bass_utils import run_bass_kernel_spmd

F32 = mybir.dt.float32
BF16 = mybir.dt.bfloat16

B, N, DIM, H, DH = 2, 2048, 1024, 16, 64
INNER = H * DH          # 1024
HC = 4                  # heads per core
E = HC * DH             # 256 inner cols per core
NT = N // 128           # 16 token tiles
CT = DIM // 128         # 8 contraction chunks
SCALE = DH ** -0.5

IB = 512                # i-block for the attention inner loop
NIB = N // IB


def build_nc():
    nc = bacc.Bacc()
    xt_ext = nc.declare_dram_parameter("xt", [DIM, N], BF16, isOutput=False)
    wq_ext = nc.declare_dram_parameter("wq", [DIM, E], BF16, isOutput=False)
    wk_ext = nc.declare_dram_parameter("wk", [DIM, E], BF16, isOutput=False)
    wv_ext = nc.declare_dram_parameter("wv", [DIM, E], BF16, isOutput=False)
    # Wout relaid as [128, 2, DIM]: row s*64+d, pair g holds
    # Wout[(2g+s)*64 + d, :] so one K=128 matmul sums a head pair.
    wout_ext = nc.declare_dram_parameter("wout", [128, 2, DIM], BF16, isOutput=False)
    vld_ext = nc.declare_dram_parameter("validT", [N, N], BF16, isOutput=False)
    out_ext = nc.declare_dram_parameter("out", [N, DIM], BF16, isOutput=True)

    Exp = mybir.ActivationFunctionType.Exp

    with tile.TileContext(nc) as tc:
        with (
            tc.tile_pool(name="persist", bufs=1) as pp,
            tc.tile_pool(name="pts", bufs=3) as pts,
            tc.tile_pool(name="norm", bufs=2) as nrm,
            tc.tile_pool(name="ostage", bufs=3) as ost,
            tc.tile_pool(name="psA", bufs=2, space="PSUM") as psA,
            tc.tile_pool(name="psB", bufs=4, space="PSUM") as psB,
        ):
            # ---- persistent SBUF tiles ----
            xt = pp.tile([128, CT, N], BF16, tag="xt")
            wq = pp.tile([128, CT, E], BF16, tag="wq")
            wk = pp.tile([128, CT, E], BF16, tag="wk")
            wv = pp.tile([128, CT, E], BF16, tag="wv")
            woP = pp.tile([128, 2, DIM], BF16, tag="woP")
            qT = [pp.tile([128, N], BF16, tag=f"qT{i}", name=f"qT{i}") for i in range(2)]
            kT = [pp.tile([128, N], BF16, tag=f"kT{i}", name=f"kT{i}") for i in range(2)]
            vaug = pp.tile([128, NT, HC, DH + 1], BF16, tag="vaug")
            outP = pp.tile([128, 2, N], BF16, tag="outP")
            vldS = pp.tile([128, NT, N], BF16, tag="vldS")

            # input DMAs; xt chunked by token-quarter so the first k0
            # projection block can start after ~1/4 of xt lands.
            nc.sync.dma_start(out=wk, in_=wk_ext.rearrange("(c p) e -> p c e", p=128))
            nc.sync.dma_start(out=wq, in_=wq_ext.rearrange("(c p) e -> p c e", p=128))
            for q in range(4):
                qs = slice(q * 512, (q + 1) * 512)
                nc.sync.dma_start(
                    out=xt[:, :, qs],
                    in_=xt_ext[:, qs].rearrange("(c p) n -> p c n", p=128))
            nc.sync.dma_start(out=wv, in_=wv_ext.rearrange("(c p) e -> p c e", p=128))
            nc.sync.dma_start(out=woP, in_=wout_ext[:, :, :])
            # validT resident in SBUF, loaded once; chunked by i-quarter so
            # segment 0's slice arrives right after xt.
            for q in range(4):
                qs = slice(q * 512, (q + 1) * 512)
                nc.sync.dma_start(
                    out=vldS[:, :, qs],
                    in_=vld_ext[:, qs].rearrange("(t p) n -> p t n", p=128))
            nc.vector.memset(vaug[:, :, :, DH:DH + 1], 1.0)

            # PE warmup: ~5us of dummy matmuls while input DMAs land, so the
            # HAM clock-gate is at 8/8 when real matmuls start.
            wrm = pp.tile([64, 64], BF16, tag="wrm")
            nc.vector.memset(wrm, 0.0)
            wps = psA.tile([128, 2, IB], F32, tag="big", name="warmps")
            for wi in range(96):
                nc.tensor.matmul(wps[0:64, 0, 0:64], wrm, wrm,
                                 start=True, stop=True)

            # ---- q/k projection blocks (emitted piecemeal) ----
            def emit_qk_block(w_sb, dst, half, iq):
                qkp = psB.tile([128, 512], F32, tag="med",
                               name=f"qkp{half}_{iq}_{dst.name if hasattr(dst,'name') else 0}")
                for c in range(CT):
                    nc.tensor.matmul(
                        qkp,
                        w_sb[:, c, half * 128:half * 128 + 128],
                        xt[:, c, iq * 512:(iq + 1) * 512],
                        start=(c == 0), stop=(c == CT - 1),
                    )
                nc.vector.tensor_copy(
                    out=dst[:, iq * 512:(iq + 1) * 512], in_=qkp)

            def emit_v(jt):
                vp = psB.tile([128, E], F32, tag="med", name=f"vp{jt}")
                for c in range(CT):
                    nc.tensor.matmul(
                        vp, xt[:, c, jt * 128:(jt + 1) * 128], wv[:, c, :],
                        start=(c == 0), stop=(c == CT - 1),
                    )
                nc.vector.tensor_copy(
                    out=vaug[:, jt, :, 0:DH],
                    in_=vp.rearrange("p (h d) -> p h d", h=HC))

            def emit_outproj_unit(ib, it, fh):
                fp = psB.tile([128, 512], F32, tag="med", name=f"fp{it}_{fh}")
                for g in range(2):
                    nc.tensor.matmul(
                        fp,
                        outP[:, g, it * 128:(it + 1) * 128],
                        woP[:, g, fh * 512:(fh + 1) * 512],
                        start=(g == 0), stop=(g == 1),
                    )
                ot = ost.tile([128, 512], BF16, tag="ot", name=f"ot{it}_{fh}")
                nc.vector.tensor_copy(out=ot, in_=fp)
                nc.sync.dma_start(
                    out=out_ext[it * 128:(it + 1) * 128, fh * 512:(fh + 1) * 512],
                    in_=ot)

            # staggered normalization of a finished segment's accumulators
            def norm_piece(seg, step, state):
                p_oa, p_ib, p_hp = seg
                p_isl = slice(p_ib * IB, (p_ib + 1) * IB)
                hh, sub = divmod(step, 3)
                if sub == 0:
                    dn = nrm.tile([1, IB], F32, tag="dn", name=f"dn{p_ib}_{p_hp}_{hh}")
                    nc.vector.tensor_copy(out=dn, in_=p_oa[hh][DH:DH + 1, :])
                    rc = nrm.tile([1, IB], F32, tag="rc", name=f"rc{p_ib}_{p_hp}_{hh}")
                    nc.vector.reciprocal_approx_fast(out=rc, in_=dn)
                    state[("rc", hh)] = rc
                elif sub == 1:
                    rp = nrm.tile([DH, IB], F32, tag="rp", name=f"rp{p_ib}_{p_hp}_{hh}")
                    nc.gpsimd.partition_broadcast(rp, state[("rc", hh)])
                    state[("rp", hh)] = rp
                else:
                    if hh == 0:
                        nc.vector.tensor_mul(
                            out=outP[0:DH, p_hp, p_isl], in0=p_oa[0][0:DH, :],
                            in1=state[("rp", 0)])
                    else:
                        stg = nrm.tile([DH, IB], BF16, tag="stg",
                                       name=f"stg{p_ib}_{p_hp}")
                        nc.vector.tensor_mul(
                            out=stg, in0=p_oa[1][0:DH, :], in1=state[("rp", 1)])
                        nc.sync.dma_start(
                            out=outP[DH:128, p_hp, p_isl], in_=stg)

            # injection schedule per segment index 0..7 (hp-major: seg = hp*4+ib)
            # v-proj for segment 0 is handled at pair-top (just-in-time for
            # the avs); everything else spreads over jt slots 6..15, after
            # the pending norm of the previous segment has freed its oa
            # slots and written its outP range.
            inject = {k: [] for k in range(8)}
            inject[0] = [("qk", "q0", 1)]
            inject[1] = [("qk", "k1", 0), ("qk", "k1", 1), ("qk", "k1", 2),
                         ("qk", "k1", 3), ("qk", "q0", 2)]
            inject[2] = [("qk", "q1", 0), ("qk", "q0", 3)]
            inject[3] = [("qk", "q1", 1)]
            inject[4] = [("qk", "q1", 2)]
            inject[5] = [("op", 0), ("qk", "q1", 3)]
            inject[6] = [("op", 1)]
            inject[7] = [("op", 2)]

            qk_map = {
                "q0": (wq, qT[0], 0), "k0": (wk, kT[0], 0),
                "q1": (wq, qT[1], 1), "k1": (wk, kT[1], 1),
            }

            # prologue: k0 fully + q0 block 0 -> segment 0 can start
            for iq in range(4):
                emit_qk_block(wk, kT[0], 0, iq)
            emit_qk_block(wq, qT[0], 0, 0)

            pending = None
            pending_state = {}
            pending_step = 0

            for seg in range(8):
                hp, ib = divmod(seg, 4)
                isl = slice(ib * IB, (ib + 1) * IB)
                inj = list(inject[seg])
                # expand "op" entries into 8 outproj units, "v" kept as is
                units = []
                for item in inj:
                    if item[0] == "op":
                        oib = item[1]
                        for it in range(oib * 4, oib * 4 + 4):
                            for fh in range(2):
                                units.append(("op1", oib, it, fh))
                    else:
                        units.append(item)
                # spread units over jt slots 6..15 (after pending-norm)
                slots = {}
                for u_i, u in enumerate(units):
                    slot = 6 + min(9, (u_i * 10) // max(1, len(units)))
                    slots.setdefault(slot, []).append(u)

                oa = [psB.tile([DH + 1, IB], F32, tag="med",
                               name=f"oa{seg}_{i}") for i in range(2)]
                for jp in range(NT // 2):  # 8 jt pairs
                    je, jo = 2 * jp, 2 * jp + 1
                    if seg == 0:
                        # just-in-time v projection: avs of this pair need
                        # vaug[:, je/jo] and read it in program order.
                        emit_v(je)
                        emit_v(jo)
                    pt2 = pts.tile([128, 2, 2, IB], BF16, tag="pt",
                                   name=f"pt{seg}_{jp}")
                    ptm2 = pts.tile([128, 2, 2, IB], BF16, tag="ptm",
                                    name=f"ptm{seg}_{jp}")
                    for jt in (je, jo):
                        st = psA.tile([128, 2, IB], F32, tag="big",
                                      name=f"st{seg}_{jt}")
                        for hh in range(2):
                            q_rows = slice(hh * 64, hh * 64 + 64)
                            nc.tensor.matmul(
                                st[:, hh, :],
                                kT[hp][q_rows, jt * 128:(jt + 1) * 128],
                                qT[hp][q_rows, isl],
                                start=True, stop=True,
                                tile_position=(hh * 64, 0),
                            )
                        nc.scalar.activation(out=pt2[:, jt % 2, :, :], in_=st,
                                             func=Exp, scale=SCALE)
                    nc.vector.tensor_mul(
                        out=ptm2, in0=pt2,
                        in1=vldS[:, je:je + 2, isl].unsqueeze(2)
                            .broadcast_to((128, 2, 2, IB)))
                    for jt in (je, jo):
                        for hh in range(2):
                            nc.tensor.matmul(
                                oa[hh][:, :],
                                vaug[:, jt, 2 * hp + hh, :],
                                ptm2[:, jt % 2, hh, :],
                                start=(jt == 0), stop=(jt == NT - 1),
                            )
                    # staggered pieces: pending norm first, then injections
                    for jt in (je, jo):
                        if pending is not None and pending_step < 6:
                            norm_piece(pending, pending_step, pending_state)
                            pending_step += 1
                            if pending_step == 6:
                                pending = None
                        for u in slots.get(jt, []):
                            if u[0] == "v":
                                emit_v(u[1])
                            elif u[0] == "qk":
                                w_sb, dst, half = qk_map[u[1]]
                                emit_qk_block(w_sb, dst, half, u[2])
                            elif u[0] == "op1":
                                emit_outproj_unit(u[1], u[2], u[3])
                pending = (oa, ib, hp)
                pending_state = {}
                pending_step = 0

            # tail: final segment's norm + last outproj block
            while pending_step < 6:
                norm_piece(pending, pending_step, pending_state)
                pending_step += 1
            for it in range(12, 16):
                for fh in range(2):
                    emit_outproj_unit(3, it, fh)

    nc.finalize()
    return nc


_NC = None


def _get_nc():
    global _NC
    if _NC is None:
        _NC = build_nc()
    return _NC


def _install_trace_shim():
    """Provide antenv.axon_hooks for NTFF profiling under axon."""
    import types
    try:
        import antenv.axon_hooks  # noqa: F401
        return True
    except ImportError:
        pass
    try:
        from trn_agent_boot.trn_boot import _ntff_profile_via_ctypes
        hook = _ntff_profile_via_ctypes("/opt/axon/libaxon_pjrt.so")
    except Exception:
        return False
    if hook is None:
        return False
    mod = types.ModuleType("antenv.axon_hooks")
    mod.get_axon_ntff_profile_hook = lambda: hook
    sys.modules["antenv.axon_hooks"] = mod
    return True


def kernel(x, Wq, Wkv, Wout, attn_mask, key_padding_mask, _trace=False):
    x = np.asarray(x, dtype=np.float32)
    Wq = np.asarray(Wq, dtype=np.float32)
    Wkv = np.asarray(Wkv, dtype=np.float32)
    Wout = np.asarray(Wout, dtype=np.float32)
    attn_mask = np.asarray(attn_mask, dtype=bool)
    key_padding_mask = np.asarray(key_padding_mask, dtype=bool)

    nc = _get_nc()

    xT = [np.ascontiguousarray(x[b].T).astype(ml_dtypes.bfloat16) for b in range(B)]
    validT = []
    for b in range(B):
        v = ~(attn_mask.T | key_padding_mask[b][:, None])
        validT.append(v.astype(ml_dtypes.bfloat16))
    wq_s, wk_s, wv_s, wo_s = [], [], [], []
    for g in range(4):  # 4 head groups
        cols = slice(g * E, (g + 1) * E)
        wq_s.append(np.ascontiguousarray(Wq[:, cols]).astype(ml_dtypes.bfloat16))
        wk_s.append(np.ascontiguousarray(Wkv[:, cols]).astype(ml_dtypes.bfloat16))
        wv_s.append(np.ascontiguousarray(Wkv[:, INNER + g * E: INNER + (g + 1) * E]).astype(ml_dtypes.bfloat16))
        # [128, 2, DIM]: row s*64+d of pair g = Wout[(2g+s)*64+d, :]
        w4 = Wout[cols, :].reshape(2, 2, DH, DIM)  # [g, s, d, f]
        wo_s.append(np.ascontiguousarray(
            w4.transpose(1, 2, 0, 3).reshape(128, 2, DIM)).astype(ml_dtypes.bfloat16))

    in_maps = []
    for c in range(8):
        b, g = c // 4, c % 4
        in_maps.append({
            "xt": xT[b], "wq": wq_s[g], "wk": wk_s[g], "wv": wv_s[g],
            "wout": wo_s[g], "validT": validT[b],
        })

    if _trace:
        _install_trace_shim()
    res = run_bass_kernel_spmd(nc, in_maps, core_ids=list(range(8)), trace=_trace)

    out = np.empty((B, N, DIM), dtype=np.float32)
    for b in range(B):
        acc = res.results[4 * b]["out"].astype(np.float32)
        for g in range(1, 4):
            acc = acc + res.results[4 * b + g]["out"].astype(np.float32)
        out[b] = acc
    if _trace:
        kernel.last_exec_time_ns = res.exec_time_ns
    return out
